# revision 4
# baseline (speedup 1.0000x reference)
"""RegionLoss (YOLO-style pose loss) on 8 Trainium2 NeuronCores.

Strategy: pure data parallel over the batch (16 images/core). The loss is
extremely sparse: the x/y terms and all metrics only touch the <=5 GT cells
per image, so each core does
  * a dense pass over just the conf channel (sigmoid -> sum sq, count>0.5),
  * one indirect-DMA gather of the 19 needed channel values at each GT cell
    (host passes a channels-last copy so each cell's channels are contiguous),
  * tiny per-GT vector math + a PE column-reduction to 7 partial scalars.
Host sums the 8x7 partials (the psum step) and assembles the outputs.
"""
import numpy as np
import concourse.bass as bass
import concourse.tile as tile
from concourse import bacc, mybir
from concourse.bass_utils import run_bass_kernel_spmd

K = 9
NH = NW = 76
HW = NH * NW              # 5776
NBC = 16                  # images per core
MAXGT = 5
NG = NBC * MAXGT          # 80
C = 2 * K + 1 + 1         # 20 channels
NCH = 2 * K + 1           # 19 gathered channels (class ch unused)
IMG_STRIDE = C * HW       # 115520 elements per image (either layout)
P = 128
FREE = NBC * HW // P      # 722
NCORES = 8
B = NBC * NCORES          # 128
AX = (640.0 / 76.0) ** 2
AY = (480.0 / 76.0) ** 2
CC9 = float(1.0 / (np.exp(2.0) - 1.0 + 1e-5) / 9.0)

F32 = mybir.dt.float32
I32 = mybir.dt.int32


def build_nc():
    nc = bacc.Bacc(None)
    confb = nc.dram_tensor("confb", [P, FREE], F32, kind="ExternalInput")
    xoutt = nc.dram_tensor("xoutt", [NBC, NH, NW, C], F32, kind="ExternalInput")
    tgtp = nc.dram_tensor("tgtp", [NG, 2 * K], F32, kind="ExternalInput")
    bofs = nc.dram_tensor("bofs", [NG, 1], F32, kind="ExternalInput")
    partials = nc.dram_tensor("partials", [7, 1], F32, kind="ExternalOutput")

    with tile.TileContext(nc) as tc:
        with tc.tile_pool(name="sb", bufs=1) as sb, \
             tc.tile_pool(name="ps", bufs=1, space="PSUM") as ps:

            # ---------------- dense conf branch ----------------
            conf_t = sb.tile([P, FREE], F32)
            nc.sync.dma_start(out=conf_t[:], in_=confb[:])

            dvals = sb.tile([P, 2], F32)
            sigz = sb.tile([P, FREE], F32)
            nc.scalar.activation(out=sigz[:], in_=conf_t[:], func=mybir.ActivationFunctionType.Sigmoid)
            junk_d = sb.tile([P, FREE], F32)
            nc.scalar.activation(out=junk_d[:], in_=sigz[:],
                                 func=mybir.ActivationFunctionType.Square,
                                 accum_out=dvals[:, 0:1])
            junk_c = sb.tile([P, FREE], F32)
            nc.vector.tensor_scalar(
                out=junk_c[:], in0=conf_t[:], scalar1=0.0, scalar2=None,
                op0=mybir.AluOpType.is_gt, op1=mybir.AluOpType.add,
                accum_out=dvals[:, 1:2])

            ones = sb.tile([P, 1], F32)
            nc.vector.memset(ones[:], 1.0)
            psum_d = ps.tile([2, 1], F32)
            nc.tensor.matmul(out=psum_d[:], lhsT=dvals[:], rhs=ones[:], start=True, stop=True)

            # ---------------- target processing ----------------
            tgt_t = sb.tile([NG, 2 * K], F32)
            nc.sync.dma_start(out=tgt_t[:], in_=tgtp[:])
            bofs_t = sb.tile([NG, 1], F32)
            nc.sync.dma_start(out=bofs_t[:], in_=bofs[:])

            # valid: [16,5] cumprod then reshape to [128,1] weights
            nz16 = sb.tile([NBC, MAXGT], F32)
            tgt5 = sb.tile([NBC, MAXGT], F32)
            nc.sync.dma_start(out=tgt5[:], in_=tgtp[:, 0:1])
            nc.vector.tensor_scalar(out=nz16[:], in0=tgt5[:], scalar1=0.0, scalar2=None,
                                    op0=mybir.AluOpType.not_equal)
            for t in range(1, MAXGT):
                nc.vector.tensor_tensor(out=nz16[:, t:t + 1], in0=nz16[:, t - 1:t],
                                        in1=nz16[:, t:t + 1], op=mybir.AluOpType.mult)
            valid_w = sb.tile([P, 1], F32)
            nc.vector.memset(valid_w[:], 0.0)
            nc.sync.dma_start(out=valid_w[0:NG, 0:1], in_=nz16[:])

            # gx, gy
            gx = sb.tile([NG, K], F32)
            gy = sb.tile([NG, K], F32)
            nc.vector.tensor_scalar_mul(out=gx[:], in0=tgt_t[:, 0:K], scalar1=float(NW))
            nc.vector.tensor_scalar_mul(out=gy[:], in0=tgt_t[:, K:2 * K], scalar1=float(NH))

            # floor(gx[:,0]) robust to convert rounding mode
            def floorcol(src_ap, name):
                ci = sb.tile([NG, 1], I32, tag=f"{name}_i")
                cf = sb.tile([NG, 1], F32, tag=f"{name}_f")
                fx = sb.tile([NG, 1], F32, tag=f"{name}_x")
                nc.vector.tensor_copy(out=ci[:], in_=src_ap)
                nc.vector.tensor_copy(out=cf[:], in_=ci[:])
                nc.vector.tensor_tensor(out=fx[:], in0=cf[:], in1=src_ap,
                                        op=mybir.AluOpType.is_gt)
                nc.vector.tensor_tensor(out=cf[:], in0=cf[:], in1=fx[:],
                                        op=mybir.AluOpType.subtract)
                return cf

            cxf = floorcol(gx[:, 0:1], "cx")
            cyf = floorcol(gy[:, 0:1], "cy")

            # idx = (cyf*76 + cxf)*20 + bofs  (channels-last element offsets; f32 exact)
            basef = sb.tile([NG, 1], F32)
            nc.vector.tensor_scalar(out=basef[:], in0=cyf[:], scalar1=float(NW),
                                    scalar2=None, op0=mybir.AluOpType.mult)
            nc.vector.tensor_tensor(out=basef[:], in0=basef[:], in1=cxf[:],
                                    op=mybir.AluOpType.add)
            idxf = sb.tile([NG, 1], F32)
            nc.vector.scalar_tensor_tensor(out=idxf[:], in0=basef[:], scalar=float(C),
                                           in1=bofs_t[:], op0=mybir.AluOpType.mult,
                                           op1=mybir.AluOpType.add)
            idx = sb.tile([NG, 1], I32)
            nc.vector.tensor_copy(out=idx[:], in_=idxf[:])

            # indirect gather: g_t[g, :] = xoutt.flat[idx[g] : idx[g]+19]
            # (HW semantics: one index per partition, contiguous run per index)
            g_t = sb.tile([NG, NCH], F32)
            xflat = xoutt[:].rearrange("b h w c -> b (h w c)")
            nc.gpsimd.indirect_dma_start(
                out=g_t[:], out_offset=None, in_=xflat,
                in_offset=bass.IndirectOffsetOnAxis(ap=idx[:], axis=1))

            # channels-last layout: col 2k = x_k, col 2k+1 = y_k, col 18 = conf
            nc.scalar.activation(out=g_t[:, 0:2], in_=g_t[:, 0:2],
                                 func=mybir.ActivationFunctionType.Sigmoid)
            sc = sb.tile([NG, 1], F32)
            nc.scalar.activation(out=sc[:], in_=g_t[:, 2 * K:2 * K + 1],
                                 func=mybir.ActivationFunctionType.Sigmoid)

            # tx/ty, dx/dy, per-GT sums
            gvals = sb.tile([P, 5], F32)
            nc.vector.memset(gvals[:], 0.0)
            tx = sb.tile([NG, K], F32)
            ty = sb.tile([NG, K], F32)
            nc.vector.tensor_scalar(out=tx[:], in0=gx[:], scalar1=cxf[:, 0:1], scalar2=None,
                                    op0=mybir.AluOpType.subtract)
            nc.vector.tensor_scalar(out=ty[:], in0=gy[:], scalar1=cyf[:, 0:1], scalar2=None,
                                    op0=mybir.AluOpType.subtract)
            dx = sb.tile([NG, K], F32)
            dy = sb.tile([NG, K], F32)
            nc.vector.tensor_tensor(out=dx[:], in0=g_t[:, 0:2 * K:2], in1=tx[:],
                                    op=mybir.AluOpType.subtract)
            nc.vector.tensor_tensor(out=dy[:], in0=g_t[:, 1:2 * K + 1:2], in1=ty[:],
                                    op=mybir.AluOpType.subtract)
            dx2 = sb.tile([NG, K], F32)
            dy2 = sb.tile([NG, K], F32)
            nc.vector.scalar_tensor_tensor(
                out=dx2[:], in0=dx[:], scalar=1.0, in1=dx[:],
                op0=mybir.AluOpType.mult, op1=mybir.AluOpType.mult,
                accum_out=gvals[0:NG, 0:1])
            nc.vector.scalar_tensor_tensor(
                out=dy2[:], in0=dy[:], scalar=1.0, in1=dy[:],
                op0=mybir.AluOpType.mult, op1=mybir.AluOpType.mult,
                accum_out=gvals[0:NG, 1:2])

            # corner confidence
            dy2b = sb.tile([NG, K], F32)
            nc.vector.tensor_scalar_mul(out=dy2b[:], in0=dy2[:], scalar1=AY)
            s2 = sb.tile([NG, K], F32)
            nc.vector.scalar_tensor_tensor(out=s2[:], in0=dx2[:], scalar=AX, in1=dy2b[:],
                                           op0=mybir.AluOpType.mult,
                                           op1=mybir.AluOpType.add)
            dd = sb.tile([NG, K], F32)
            nc.scalar.activation(out=dd[:], in_=s2[:], func=mybir.ActivationFunctionType.Sqrt)
            bias2 = sb.tile([NG, 1], F32)
            nc.vector.memset(bias2[:], 2.0)
            ee = sb.tile([NG, K], F32)
            nc.scalar.activation(out=ee[:], in_=dd[:], func=mybir.ActivationFunctionType.Exp,
                                 bias=bias2[:], scale=-1.0 / 40.0)
            mm = sb.tile([NG, K], F32)
            nc.vector.tensor_scalar(out=mm[:], in0=dd[:], scalar1=80.0, scalar2=None,
                                    op0=mybir.AluOpType.is_lt)
            ce = sb.tile([NG, K], F32)
            nc.vector.tensor_scalar(out=ce[:], in0=ee[:], scalar1=1.0, scalar2=CC9,
                                    op0=mybir.AluOpType.subtract, op1=mybir.AluOpType.mult)
            junk_g = sb.tile([NG, K], F32)
            confgt = sb.tile([NG, 1], F32)
            nc.vector.scalar_tensor_tensor(
                out=junk_g[:], in0=ce[:], scalar=1.0, in1=mm[:],
                op0=mybir.AluOpType.mult, op1=mybir.AluOpType.mult,
                accum_out=confgt[:])
            nc.vector.tensor_scalar(out=gvals[0:NG, 4:5], in0=confgt[:], scalar1=0.7,
                                    scalar2=None, op0=mybir.AluOpType.is_gt)

            # conf correction 1 - 2*sigma(conf_logit)
            nc.vector.tensor_scalar(out=gvals[0:NG, 2:3], in0=sc[:], scalar1=-2.0,
                                    scalar2=1.0, op0=mybir.AluOpType.mult,
                                    op1=mybir.AluOpType.add)
            nc.vector.memset(gvals[0:NG, 3:4], 1.0)

            psum_g = ps.tile([5, 1], F32)
            nc.tensor.matmul(out=psum_g[:], lhsT=gvals[:], rhs=valid_w[:], start=True, stop=True)

            res_g = sb.tile([5, 1], F32)
            res_d = sb.tile([2, 1], F32)
            nc.vector.tensor_copy(out=res_g[:], in_=psum_g[:])
            nc.vector.tensor_copy(out=res_d[:], in_=psum_d[:])
            nc.sync.dma_start(out=partials[0:5, 0:1], in_=res_g[:])
            nc.sync.dma_start(out=partials[5:7, 0:1], in_=res_d[:])
    nc.compile()
    return nc


def host_shards(output, target):
    """Split full inputs into per-core input maps (layout only, no math)."""
    output = np.ascontiguousarray(np.asarray(output, dtype=np.float32))
    target = np.ascontiguousarray(np.asarray(target, dtype=np.float32))
    g = np.arange(NG)
    bofs = ((g[:, None] // MAXGT) * IMG_STRIDE).astype(np.float32)
    maps = []
    for i in range(NCORES):
        ob = output[i * NBC:(i + 1) * NBC]
        confb = np.ascontiguousarray(ob[:, 2 * K].reshape(P, FREE))
        xoutt = np.ascontiguousarray(ob.transpose(0, 2, 3, 1))
        tb = target[i * NBC:(i + 1) * NBC].reshape(NBC, MAXGT, 2 * K + 3)
        tp = np.ascontiguousarray(
            np.concatenate([tb[:, :, 1:2 * K + 1:2], tb[:, :, 2:2 * K + 2:2]], axis=2)
            .reshape(NG, 2 * K))
        maps.append({"confb": confb, "xoutt": xoutt, "tgtp": tp, "bofs": bofs})
    return maps


def combine(partials_list):
    p = np.stack([np.asarray(q).reshape(7) for q in partials_list]).sum(
        axis=0, dtype=np.float64).astype(np.float32)
    loss_x, loss_y, corr, ngt_cnt, ncorr_cnt, sqsum, prop_cnt = [np.float32(v) for v in p]
    loss_conf = np.float32(sqsum + corr)
    loss = np.float32(np.float32(loss_x + loss_y) + loss_conf)
    nB = np.float32(B)
    return (loss, np.float32(ngt_cnt / nB), np.float32(ncorr_cnt / nB),
            np.float32(prop_cnt / nB), loss_x, loss_y, loss_conf)


_NC_CACHE = None


def _get_nc():
    global _NC_CACHE
    if _NC_CACHE is None:
        _NC_CACHE = build_nc()
    return _NC_CACHE


def kernel(output, target):
    nc = _get_nc()
    maps = host_shards(output, target)
    res = run_bass_kernel_spmd(nc, maps, core_ids=list(range(NCORES)))
    parts = [res.results[i]["partials"] for i in range(NCORES)]
    return combine(parts)
